# revision 6
# baseline (speedup 1.0000x reference)
"""CrossEntropyWithZLoss fused loss kernel for 8x Trainium2 NeuronCores.

Strategy (data parallel over tokens, per the sharding hint):
  - Full logits [8192, 32000] f32 are sharded along N across 8 cores
    (1024 rows each).
  - Each core streams its 131 MB shard from HBM exactly once
    (memory-roofline bound ~360 GB/s/core):
      * ScalarE: activation(Exp) with fused accum_out -> per-row sum(exp(x))
      * VectorE: tensor_reduce(add)                   -> per-row sum(x)
      * GpSimd indirect DMA gather                    -> per-row x[r, target[r]]
  - Per-row partials ([128, 24] f32 per core) come back to host, which
    finishes the O(N) scalar math (lse=log(sumexp), nll, smoothing, z-loss,
    masking and the mean) and the 8-way reduction.
"""

import sys

import numpy as np

sys.path.insert(0, "/opt/trn_rl_repo")

N, V = 8192, 32000
N_CORES = 8
R = N // N_CORES  # rows per core
P = 128  # SBUF partitions
RB = R // P  # row-blocks per core
FD = 4000  # vocab-tile width (f32 -> 2 MB per DMA)
LSE_SQUARE_SCALE = 1e-4
LABEL_SMOOTHING = 0.1
IGNORE_INDEX = -100

_NC_CACHE: dict = {}


def build_bass(r=R, v=V, fd=FD):
    """Build + compile the per-core Bass program (SPMD: same program on all
    cores, each with its own input shard)."""
    import concourse.bass as bass
    import concourse.tile as tile
    from concourse import bacc, mybir

    rb_n = r // P
    nt = v // fd
    assert r % P == 0 and v % fd == 0

    nc = bacc.Bacc("TRN2", target_bir_lowering=False, debug=False)

    logits_t = nc.dram_tensor("logits", [r, v], mybir.dt.float32, kind="ExternalInput")
    offs_t = nc.dram_tensor("offsets", [P, rb_n], mybir.dt.int32, kind="ExternalInput")
    out_t = nc.dram_tensor("out", [P, 3 * rb_n], mybir.dt.float32, kind="ExternalOutput")

    logits = logits_t.ap()
    flat = logits.rearrange("r v -> (r v)").unsqueeze(1)

    with tile.TileContext(nc) as tc:
        with (
            tc.tile_pool(name="inp", bufs=6) as in_pool,
            tc.tile_pool(name="scratch", bufs=2) as scratch_pool,
            tc.tile_pool(name="stats", bufs=2) as stats_pool,
            tc.tile_pool(name="small", bufs=1) as small_pool,
        ):
            offs_tile = small_pool.tile([P, rb_n], mybir.dt.int32)
            nc.sync.dma_start(out=offs_tile[:], in_=offs_t.ap())

            res_tile = small_pool.tile([P, 3 * rb_n], mybir.dt.float32)

            # Gather x[r, target[r]] one element per row via indirect DMA.
            for rb in range(rb_n):
                nc.gpsimd.indirect_dma_start(
                    out=res_tile[:, 2 * rb_n + rb : 2 * rb_n + rb + 1],
                    out_offset=None,
                    in_=flat,
                    in_offset=bass.IndirectOffsetOnAxis(
                        ap=offs_tile[:, rb : rb + 1], axis=0
                    ),
                )

            for rb in range(rb_n):
                se_cols = stats_pool.tile([P, nt], mybir.dt.float32, tag="se")
                sx_cols = stats_pool.tile([P, nt], mybir.dt.float32, tag="sx")
                for it in range(nt):
                    x = in_pool.tile([P, fd], mybir.dt.float32, tag="x")
                    nc.sync.dma_start(
                        out=x[:],
                        in_=logits[rb * P : (rb + 1) * P, it * fd : (it + 1) * fd],
                    )
                    e = scratch_pool.tile([P, fd], mybir.dt.float32, tag="e")
                    nc.scalar.activation(
                        out=e[:],
                        in_=x[:],
                        func=mybir.ActivationFunctionType.Exp,
                        accum_out=se_cols[:, it : it + 1],
                    )
                    nc.vector.tensor_reduce(
                        out=sx_cols[:, it : it + 1],
                        in_=x[:],
                        axis=mybir.AxisListType.X,
                        op=mybir.AluOpType.add,
                    )
                nc.vector.tensor_reduce(
                    out=res_tile[:, rb : rb + 1],
                    in_=se_cols[:],
                    axis=mybir.AxisListType.X,
                    op=mybir.AluOpType.add,
                )
                nc.vector.tensor_reduce(
                    out=res_tile[:, rb_n + rb : rb_n + rb + 1],
                    in_=sx_cols[:],
                    axis=mybir.AxisListType.X,
                    op=mybir.AluOpType.add,
                )

            nc.sync.dma_start(out=out_t.ap(), in_=res_tile[:])

    nc.compile()
    return nc


def _get_nc():
    if "nc" not in _NC_CACHE:
        _NC_CACHE["nc"] = build_bass()
    return _NC_CACHE["nc"]


def _make_in_maps(logits, tgt_safe):
    """Shard rows across cores; offsets laid out [P, RB] with shard row
    rb*128+p at [p, rb] (matches device output layout)."""
    rows_local = np.arange(R, dtype=np.int64)
    in_maps = []
    for c in range(N_CORES):
        shard = np.ascontiguousarray(logits[c * R : (c + 1) * R])
        t = tgt_safe[c * R : (c + 1) * R]
        offs = (rows_local * V + t).astype(np.int32).reshape(RB, P).T
        in_maps.append({"logits": shard, "offsets": np.ascontiguousarray(offs)})
    return in_maps


def _unpack(outs):
    """Per-core [P, 3*RB] -> full [N] vectors (sum_exp, sum_x, tgt_logit)."""
    sum_exp = np.concatenate([o[:, 0:RB].T.reshape(-1) for o in outs])
    sum_x = np.concatenate([o[:, RB : 2 * RB].T.reshape(-1) for o in outs])
    tgt_l = np.concatenate([o[:, 2 * RB : 3 * RB].T.reshape(-1) for o in outs])
    return sum_exp, sum_x, tgt_l


def _finish(sum_exp, sum_x, tgt_l, mask):
    sum_exp = sum_exp.astype(np.float32)
    lse = np.log(sum_exp)
    nll = lse - tgt_l.astype(np.float32)
    smooth = lse - sum_x.astype(np.float32) / np.float32(V)
    ce_per = np.float32(1.0 - LABEL_SMOOTHING) * nll + np.float32(LABEL_SMOOTHING) * smooth
    ce_per = np.where(mask, ce_per, np.float32(0.0))
    z_per = np.where(mask, np.float32(LSE_SQUARE_SCALE) * lse * lse, np.float32(0.0))
    n_valid = np.float32(mask.sum())
    ce_loss = np.float32(np.sum(ce_per, dtype=np.float32) / n_valid)
    z_loss = np.float32(np.sum(z_per, dtype=np.float32) / n_valid)
    return np.float32(ce_loss + z_loss)


def run_device(inputs, trace=False):
    """Run the SPMD kernel on cores 0-7. Returns (loss, exec_time_ns|None)."""
    from concourse.bass_utils import run_bass_kernel_spmd

    logits = np.asarray(inputs["logits"], dtype=np.float32)
    targets = np.asarray(inputs["targets"])
    assert logits.shape == (N, V), logits.shape

    mask = targets != IGNORE_INDEX
    tgt_safe = np.where(mask, targets, 0).astype(np.int64)

    nc = _get_nc()
    in_maps = _make_in_maps(logits, tgt_safe)
    res = run_bass_kernel_spmd(nc, in_maps, list(range(N_CORES)), trace=trace)
    outs = [r["out"] for r in res.results]
    sum_exp, sum_x, tgt_l = _unpack(outs)
    loss = _finish(sum_exp, sum_x, tgt_l, mask)
    return loss, res.exec_time_ns


def kernel(logits, targets):
    loss, _ = run_device({"logits": logits, "targets": targets})
    return loss


# revision 7
# speedup vs baseline: 1.0389x; 1.0389x over previous
"""CrossEntropyWithZLoss fused loss kernel for 8x Trainium2 NeuronCores.

Strategy (data parallel over tokens, per the sharding hint):
  - Full logits [8192, 32000] f32 are sharded along N across 8 cores
    (1024 rows each).
  - Each core streams its 131 MB shard from HBM exactly once
    (memory-roofline bound ~360 GB/s/core):
      * ScalarE: activation(Exp) with fused accum_out -> per-row sum(exp(x))
      * VectorE: tensor_reduce(add)                   -> per-row sum(x)
      * GpSimd indirect DMA gather                    -> per-row x[r, target[r]]
  - Per-row partials ([128, 24] f32 per core) come back to host, which
    finishes the O(N) scalar math (lse=log(sumexp), nll, smoothing, z-loss,
    masking and the mean) and the 8-way reduction.
"""

import sys

import numpy as np

sys.path.insert(0, "/opt/trn_rl_repo")

N, V = 8192, 32000
N_CORES = 8
R = N // N_CORES  # rows per core
P = 128  # SBUF partitions
RB = R // P  # row-blocks per core
FD = 4000  # vocab-tile width (f32 -> 2 MB per DMA)
LSE_SQUARE_SCALE = 1e-4
LABEL_SMOOTHING = 0.1
IGNORE_INDEX = -100

_NC_CACHE: dict = {}


def build_bass(r=R, v=V, fd=FD):
    """Build + compile the per-core Bass program (SPMD: same program on all
    cores, each with its own input shard)."""
    import concourse.bass as bass
    import concourse.tile as tile
    from concourse import bacc, mybir

    rb_n = r // P
    nt = v // fd
    assert r % P == 0 and v % fd == 0

    nc = bacc.Bacc("TRN2", target_bir_lowering=False, debug=False)

    logits_t = nc.dram_tensor("logits", [r, v], mybir.dt.float32, kind="ExternalInput")
    offs_t = nc.dram_tensor("offsets", [P, rb_n], mybir.dt.int32, kind="ExternalInput")
    out_t = nc.dram_tensor("out", [P, 3 * rb_n], mybir.dt.float32, kind="ExternalOutput")

    logits = logits_t.ap()
    flat = logits.rearrange("r v -> (r v)").unsqueeze(1)

    with tile.TileContext(nc) as tc:
        with (
            tc.tile_pool(name="inp", bufs=6) as in_pool,
            tc.tile_pool(name="scratch", bufs=2) as scratch_pool,
            tc.tile_pool(name="stats", bufs=2) as stats_pool,
            tc.tile_pool(name="small", bufs=1) as small_pool,
        ):
            offs_tile = small_pool.tile([P, rb_n], mybir.dt.int32)
            nc.sync.dma_start(out=offs_tile[:], in_=offs_t.ap())

            res_tile = small_pool.tile([P, 3 * rb_n], mybir.dt.float32)

            # Gather x[r, target[r]] one element per row via indirect DMA.
            for rb in range(rb_n):
                nc.gpsimd.indirect_dma_start(
                    out=res_tile[:, 2 * rb_n + rb : 2 * rb_n + rb + 1],
                    out_offset=None,
                    in_=flat,
                    in_offset=bass.IndirectOffsetOnAxis(
                        ap=offs_tile[:, rb : rb + 1], axis=0
                    ),
                )

            for rb in range(rb_n):
                se_cols = stats_pool.tile([P, nt], mybir.dt.float32, tag="se")
                sx_cols = stats_pool.tile([P, nt], mybir.dt.float32, tag="sx")
                for it in range(nt):
                    x = in_pool.tile([P, fd], mybir.dt.float32, tag="x")
                    nc.sync.dma_start(
                        out=x[:],
                        in_=logits[rb * P : (rb + 1) * P, it * fd : (it + 1) * fd],
                    )
                    e = scratch_pool.tile([P, fd], mybir.dt.float32, tag="e")
                    nc.scalar.activation(
                        out=e[:],
                        in_=x[:],
                        func=mybir.ActivationFunctionType.Exp,
                        accum_out=se_cols[:, it : it + 1],
                    )
                    nc.vector.tensor_reduce(
                        out=sx_cols[:, it : it + 1],
                        in_=x[:],
                        axis=mybir.AxisListType.X,
                        op=mybir.AluOpType.add,
                    )
                nc.vector.tensor_reduce(
                    out=res_tile[:, rb : rb + 1],
                    in_=se_cols[:],
                    axis=mybir.AxisListType.X,
                    op=mybir.AluOpType.add,
                )
                nc.vector.tensor_reduce(
                    out=res_tile[:, rb_n + rb : rb_n + rb + 1],
                    in_=sx_cols[:],
                    axis=mybir.AxisListType.X,
                    op=mybir.AluOpType.add,
                )

            nc.sync.dma_start(out=out_t.ap(), in_=res_tile[:])

    nc.compile()
    return nc


def _get_nc():
    if "nc" not in _NC_CACHE:
        _NC_CACHE["nc"] = build_bass()
    return _NC_CACHE["nc"]


def _make_in_maps(logits, tgt_safe):
    """Shard rows across cores; offsets laid out [P, RB] with shard row
    rb*128+p at [p, rb] (matches device output layout)."""
    rows_local = np.arange(R, dtype=np.int64)
    in_maps = []
    for c in range(N_CORES):
        shard = np.ascontiguousarray(logits[c * R : (c + 1) * R])
        t = tgt_safe[c * R : (c + 1) * R]
        offs = (rows_local * V + t).astype(np.int32).reshape(RB, P).T
        in_maps.append({"logits": shard, "offsets": np.ascontiguousarray(offs)})
    return in_maps


def _unpack(outs):
    """Per-core [P, 3*RB] -> full [N] vectors (sum_exp, sum_x, tgt_logit)."""
    sum_exp = np.concatenate([o[:, 0:RB].T.reshape(-1) for o in outs])
    sum_x = np.concatenate([o[:, RB : 2 * RB].T.reshape(-1) for o in outs])
    tgt_l = np.concatenate([o[:, 2 * RB : 3 * RB].T.reshape(-1) for o in outs])
    return sum_exp, sum_x, tgt_l


def _finish(sum_exp, sum_x, tgt_l, mask):
    sum_exp = sum_exp.astype(np.float32)
    lse = np.log(sum_exp)
    nll = lse - tgt_l.astype(np.float32)
    smooth = lse - sum_x.astype(np.float32) / np.float32(V)
    ce_per = np.float32(1.0 - LABEL_SMOOTHING) * nll + np.float32(LABEL_SMOOTHING) * smooth
    ce_per = np.where(mask, ce_per, np.float32(0.0))
    z_per = np.where(mask, np.float32(LSE_SQUARE_SCALE) * lse * lse, np.float32(0.0))
    n_valid = np.float32(mask.sum())
    ce_loss = np.float32(np.sum(ce_per, dtype=np.float32) / n_valid)
    z_loss = np.float32(np.sum(z_per, dtype=np.float32) / n_valid)
    return np.float32(ce_loss + z_loss)


def run_device(inputs, trace=False, trace_cores=None):
    """Run the SPMD kernel on cores 0-7. Returns (loss, exec_time_ns|None)."""
    from concourse.bass_utils import run_bass_kernel_spmd

    logits = np.asarray(inputs["logits"], dtype=np.float32)
    targets = np.asarray(inputs["targets"])
    assert logits.shape == (N, V), logits.shape

    mask = targets != IGNORE_INDEX
    tgt_safe = np.where(mask, targets, 0).astype(np.int64)

    nc = _get_nc()
    in_maps = _make_in_maps(logits, tgt_safe)
    res = run_bass_kernel_spmd(
        nc, in_maps, list(range(N_CORES)), trace=trace, trace_cores=trace_cores
    )
    outs = [r["out"] for r in res.results]
    sum_exp, sum_x, tgt_l = _unpack(outs)
    loss = _finish(sum_exp, sum_x, tgt_l, mask)
    return loss, res.exec_time_ns


def kernel(logits, targets):
    loss, _ = run_device({"logits": logits, "targets": targets})
    return loss


# revision 8
# speedup vs baseline: 1.2693x; 1.2217x over previous
"""CrossEntropyWithZLoss fused loss kernel for 8x Trainium2 NeuronCores.

Strategy (data parallel over tokens, per the sharding hint):
  - Full logits [8192, 32000] f32 are sharded along N across 8 cores
    (1024 rows each).
  - Each core streams its 131 MB shard from HBM exactly once
    (memory-roofline bound ~360 GB/s/core):
      * ScalarE: activation(Exp) with fused accum_out -> per-row sum(exp(x))
      * VectorE: tensor_reduce(add)                   -> per-row sum(x)
      * GpSimd indirect DMA gather                    -> per-row x[r, target[r]]
  - Per-row partials ([128, 24] f32 per core) come back to host, which
    finishes the O(N) scalar math (lse=log(sumexp), nll, smoothing, z-loss,
    masking and the mean) and the 8-way reduction.
"""

import sys

import numpy as np

sys.path.insert(0, "/opt/trn_rl_repo")

N, V = 8192, 32000
N_CORES = 8
R = N // N_CORES  # rows per core
P = 128  # SBUF partitions
RB = R // P  # row-blocks per core
FD = 4000  # vocab-tile width (f32 -> 2 MB per DMA)
LSE_SQUARE_SCALE = 1e-4
LABEL_SMOOTHING = 0.1
IGNORE_INDEX = -100

_NC_CACHE: dict = {}


def build_bass(r=R, v=V, fd=FD):
    """Build + compile the per-core Bass program (SPMD: same program on all
    cores, each with its own input shard)."""
    import concourse.bass as bass
    import concourse.tile as tile
    from concourse import bacc, mybir

    rb_n = r // P
    nt = v // fd
    assert r % P == 0 and v % fd == 0

    nc = bacc.Bacc("TRN2", target_bir_lowering=False, debug=False)

    logits_t = nc.dram_tensor("logits", [r, v], mybir.dt.float32, kind="ExternalInput")
    offs_t = nc.dram_tensor("offsets", [P, rb_n], mybir.dt.int32, kind="ExternalInput")
    out_t = nc.dram_tensor("out", [P, 3 * rb_n], mybir.dt.float32, kind="ExternalOutput")

    logits = logits_t.ap()
    flat = logits.rearrange("r v -> (r v)").unsqueeze(1)

    with tile.TileContext(nc) as tc:
        with (
            tc.tile_pool(name="inp", bufs=6) as in_pool,
            tc.tile_pool(name="scratch", bufs=2) as scratch_pool,
            tc.tile_pool(name="stats", bufs=2) as stats_pool,
            tc.tile_pool(name="small", bufs=1) as small_pool,
        ):
            offs_tile = small_pool.tile([P, rb_n], mybir.dt.int32)
            nc.sync.dma_start(out=offs_tile[:], in_=offs_t.ap())

            res_tile = small_pool.tile([P, 3 * rb_n], mybir.dt.float32)

            # Gather x[r, target[r]] one element per row via indirect DMA.
            for rb in range(rb_n):
                nc.gpsimd.indirect_dma_start(
                    out=res_tile[:, 2 * rb_n + rb : 2 * rb_n + rb + 1],
                    out_offset=None,
                    in_=flat,
                    in_offset=bass.IndirectOffsetOnAxis(
                        ap=offs_tile[:, rb : rb + 1], axis=0
                    ),
                )

            for rb in range(rb_n):
                se_cols = stats_pool.tile([P, nt], mybir.dt.float32, tag="se")
                sx_cols = stats_pool.tile([P, nt], mybir.dt.float32, tag="sx")
                for it in range(nt):
                    x = in_pool.tile([P, fd], mybir.dt.float32, tag="x")
                    nc.sync.dma_start(
                        out=x[:],
                        in_=logits[rb * P : (rb + 1) * P, it * fd : (it + 1) * fd],
                    )
                    e = scratch_pool.tile([P, fd], mybir.dt.float32, tag="e")
                    nc.scalar.activation(
                        out=e[:],
                        in_=x[:],
                        func=mybir.ActivationFunctionType.Exp,
                        accum_out=se_cols[:, it : it + 1],
                    )
                    nc.vector.tensor_reduce(
                        out=sx_cols[:, it : it + 1],
                        in_=x[:],
                        axis=mybir.AxisListType.X,
                        op=mybir.AluOpType.add,
                    )
                nc.vector.tensor_reduce(
                    out=res_tile[:, rb : rb + 1],
                    in_=se_cols[:],
                    axis=mybir.AxisListType.X,
                    op=mybir.AluOpType.add,
                )
                nc.vector.tensor_reduce(
                    out=res_tile[:, rb_n + rb : rb_n + rb + 1],
                    in_=sx_cols[:],
                    axis=mybir.AxisListType.X,
                    op=mybir.AluOpType.add,
                )

            nc.sync.dma_start(out=out_t.ap(), in_=res_tile[:])

    nc.compile()
    return nc


def _get_nc():
    if "nc" not in _NC_CACHE:
        _NC_CACHE["nc"] = build_bass()
    return _NC_CACHE["nc"]


def _make_in_maps(logits, tgt_safe):
    """Shard rows across cores; offsets laid out [P, RB] with shard row
    rb*128+p at [p, rb] (matches device output layout)."""
    rows_local = np.arange(R, dtype=np.int64)
    in_maps = []
    for c in range(N_CORES):
        shard = np.ascontiguousarray(logits[c * R : (c + 1) * R])
        t = tgt_safe[c * R : (c + 1) * R]
        offs = (rows_local * V + t).astype(np.int32).reshape(RB, P).T
        in_maps.append({"logits": shard, "offsets": np.ascontiguousarray(offs)})
    return in_maps


def _unpack(outs):
    """Per-core [P, 3*RB] -> full [N] vectors (sum_exp, sum_x, tgt_logit)."""
    sum_exp = np.concatenate([o[:, 0:RB].T.reshape(-1) for o in outs])
    sum_x = np.concatenate([o[:, RB : 2 * RB].T.reshape(-1) for o in outs])
    tgt_l = np.concatenate([o[:, 2 * RB : 3 * RB].T.reshape(-1) for o in outs])
    return sum_exp, sum_x, tgt_l


def _finish(sum_exp, sum_x, tgt_l, mask):
    sum_exp = sum_exp.astype(np.float32)
    lse = np.log(sum_exp)
    nll = lse - tgt_l.astype(np.float32)
    smooth = lse - sum_x.astype(np.float32) / np.float32(V)
    ce_per = np.float32(1.0 - LABEL_SMOOTHING) * nll + np.float32(LABEL_SMOOTHING) * smooth
    ce_per = np.where(mask, ce_per, np.float32(0.0))
    z_per = np.where(mask, np.float32(LSE_SQUARE_SCALE) * lse * lse, np.float32(0.0))
    n_valid = np.float32(mask.sum())
    ce_loss = np.float32(np.sum(ce_per, dtype=np.float32) / n_valid)
    z_loss = np.float32(np.sum(z_per, dtype=np.float32) / n_valid)
    return np.float32(ce_loss + z_loss)


def _ensure_ntff_hook():
    """The agent image's antenv lacks axon_hooks; recreate the NTFF profile
    hook from trn_agent_boot so run_bass_kernel_spmd(trace=True) works."""
    import types

    if "antenv.axon_hooks" in sys.modules:
        return
    try:
        import importlib

        sys.path.insert(0, "/root/.axon_site")
        tb = importlib.import_module("trn_agent_boot.trn_boot")
        hook = tb._ntff_profile_via_ctypes("/opt/axon/libaxon_pjrt.so")
        mod = types.ModuleType("antenv.axon_hooks")
        mod.get_axon_ntff_profile_hook = lambda: hook
        mod.set_axon_ntff_profile_hook = lambda h: None
        sys.modules["antenv.axon_hooks"] = mod
    except Exception:
        pass


def run_device(inputs, trace=False, trace_cores=None):
    """Run the SPMD kernel on cores 0-7. Returns (loss, exec_time_ns|None)."""
    from concourse.bass_utils import run_bass_kernel_spmd

    _ensure_ntff_hook()

    logits = np.asarray(inputs["logits"], dtype=np.float32)
    targets = np.asarray(inputs["targets"])
    assert logits.shape == (N, V), logits.shape

    mask = targets != IGNORE_INDEX
    tgt_safe = np.where(mask, targets, 0).astype(np.int64)

    nc = _get_nc()
    in_maps = _make_in_maps(logits, tgt_safe)
    res = run_bass_kernel_spmd(
        nc, in_maps, list(range(N_CORES)), trace=trace, trace_cores=trace_cores
    )
    outs = [r["out"] for r in res.results]
    sum_exp, sum_x, tgt_l = _unpack(outs)
    loss = _finish(sum_exp, sum_x, tgt_l, mask)
    return loss, res.exec_time_ns


def kernel(logits, targets):
    loss, _ = run_device({"logits": logits, "targets": targets})
    return loss
